# revision 77
# baseline (speedup 1.0000x reference)
"""Paged-KV varlen causal GQA attention for Trainium2, sharded over 8 NeuronCores.

Problem (hardcoded from spec): T=4096 tokens, 16 q heads / 8 kv heads, head_dim=64,
scale=0.125. k/v are scattered into paged caches via slot_mapping, read back, and
causal varlen attention (segments from cu_seqlens) is computed.

Sharding: tensor-parallel over kv heads -- core h gets kv head h and its 2 GQA
query heads. slot_mapping / cu_seqlens handled on host (index math only).

Device kernel (per core). ScalarE exp is the bottleneck (~31us of pure exp
work per core at 1 elem/cycle/lane), so everything is structured to keep the
exp stream dense (measured ~100% ScalarE occupancy mid-stream):
  - per seg-head (unit), the 8 causal key-tile score strips (spans 1024..128)
    are bin-packed into 3 PSUM score tiles of [128,1536] f32 (3 banks each,
    2 rotating buffers) -> 3 exp instructions of FD=1536 per unit instead of
    many small ones (per-instruction overhead is ~222 ScalarE cycles). No
    matmul output may cross a 512-f32 PSUM bank boundary, so QK pieces are
    cut at bank edges.
  - ScalarE does ONLY exp; a warmup activation at t=0 absorbs the ~2.7us ACT
    table load; a dummy-matmul chain keeps the PE p-state ramped through the
    initial DMA fill; QK emission runs two bins ahead of exp.
  - query cols [0,512) accumulate normally into a [65,512] oT block (row 64 =
    softmax denominator via a ones-column in the V tile), then PSUM->SBUF
    copy + PE transpose + DVE normalize + DMA, deferred so nothing stalls
    the QK/PV cadence.
  - query cols [512,1024) are computed TRANSPOSED on the PE (out[q, d+1] =
    se.T @ [v|1], FD=65 per key tile, "chunks" of 128 queries): no copy, no
    transpose, no block-close machinery; just reciprocal+scale straight out
    of PSUM and a row-contiguous DMA. Chunks accumulate strictly one after
    another (PSUM allows one open accumulation group per 2KB zero region).
  - the final unit is computed entirely as transposed chunks with a
    {kt7}-only last exp bin and a -2000 additive causal mask applied on the
    PE (premask), so only ~220ns of matmul + one small normalize + one
    128-row DMA remain after the last exp of the kernel.
  - segment 0's input DMAs all ride the sync queue in first-use order (the
    other queues' first transfers race the launch); later segments prefetch
    via SWDGE/scalar queues one unit ahead.
"""

import os
from contextlib import ExitStack
from math import ceil

import numpy as np
import ml_dtypes

import concourse.bass as bass
import concourse.mybir as mybir
import concourse.tile as tile
from concourse import bacc
from concourse.bass_utils import run_bass_kernel_spmd
from concourse.masks import make_identity

NKV = 8
G = 2
D = 64
SCALE = 0.125
BIN_CAP = 1536  # 3 PSUM banks of f32 per score tile

# test.py pokes these for profiling
TRACE = bool(int(os.environ.get("KERNEL_TRACE", "0")))
# Optional: off-diagonal QK in fp8e4m3 DoubleRow (2 rows/cycle on the PE).
# Off by default: since the transposed-chunk PV refactor the PE is no longer
# the bottleneck, so fp8 only adds DMA traffic and noise (rel err 1.1e-2 vs
# 3.4e-3 in bf16) for no wall-clock gain.
FP8_QK = bool(int(os.environ.get("KERNEL_FP8_QK", "0")))
TCHUNK = bool(int(os.environ.get("KERNEL_TCHUNK", "1")))
PREMASK = bool(int(os.environ.get("KERNEL_PREMASK", "1")))
LAST_RESULT = None

_PROGRAM_CACHE = {}


def _pack_bins(kts):
    """First-fit-decreasing pack of key-tile score strips into <=BIN_CAP
    column bins. Only full-partition (kp==128) tiles are merged; partial
    tiles become singletons (their exp uses [:kp] rows). Returns a list of
    (used, members); each member is (kt, rel_lo, rel_hi, bin_off) covering
    score columns [c0+rel_lo, c0+rel_hi) of key tile kt."""
    order = sorted(range(len(kts)), key=lambda i: -kts[i]["span"])
    bins = []  # [ [used, [(kt, rel_lo, rel_hi, off), ...], mergeable], ... ]
    for i in order:
        span = kts[i]["span"]
        placed = False
        if kts[i]["kp"] == 128:
            for b in bins:
                if b[2] and b[0] + span <= BIN_CAP:
                    b[1].append((i, 0, span, b[0]))
                    b[0] += span
                    placed = True
                    break
        if not placed:
            bins.append([span, [(i, 0, span, 0)], kts[i]["kp"] == 128])
    # order bins so the one containing kt0 comes first (it has the block-0
    # start writer), then by first kt index
    bins.sort(key=lambda b: min(m[0] for m in b[1]))
    return [(b[0], b[1]) for b in bins]


def _build_program(T, segments, fp8_qk=FP8_QK):
    f32 = mybir.dt.float32
    bf16 = mybir.dt.bfloat16
    fp8 = mybir.dt.float8e4

    nc = bacc.Bacc(
        "TRN2",
        target_bir_lowering=False,
        debug=False,
        enable_asserts=False,
        num_devices=8,
    )
    qT_d = nc.dram_tensor("qT", [128, T], bf16, kind="ExternalInput").ap()
    kT_d = nc.dram_tensor("kT", [64, T], bf16, kind="ExternalInput").ap()
    if fp8_qk:
        # DoubleRow layouts: contraction row (p, i) <-> head dim d = 32*i + p
        qT8_d = nc.dram_tensor("qT8", [32, 4 * T], fp8, kind="ExternalInput").ap()
        kT8_d = nc.dram_tensor("kT8", [32, 2 * T], fp8, kind="ExternalInput").ap()
    v_d = nc.dram_tensor("v", [T, D], bf16, kind="ExternalInput").ap()
    o_d = nc.dram_tensor("o", [T, 2 * D], f32, kind="ExternalOutput").ap()

    with tile.TileContext(nc) as tc, ExitStack() as ctx:
        const = ctx.enter_context(tc.tile_pool(name="const", bufs=1))
        qkpool = ctx.enter_context(tc.tile_pool(name="qk", bufs=1))
        vpool = ctx.enter_context(tc.tile_pool(name="vt", bufs=3))
        sepool = ctx.enter_context(tc.tile_pool(name="se", bufs=5))
        tpool = ctx.enter_context(tc.tile_pool(name="t65", bufs=4))
        opool = ctx.enter_context(tc.tile_pool(name="osb", bufs=8))
        ps_sc = ctx.enter_context(tc.tile_pool(name="ps_sc", bufs=2, space="PSUM"))
        ps_o = ctx.enter_context(tc.tile_pool(name="ps_o", bufs=1, space="PSUM"))

        # --- warmup: trigger the ACT exp table load at t~0 on a const tile
        warm = const.tile([1, 2], bf16)
        nc.gpsimd.memset(warm, 0.0)
        nc.scalar.activation(
            warm[:1, 0:1], warm[:1, 1:2], mybir.ActivationFunctionType.Exp
        )

        qT = qkpool.tile([128, T], bf16)
        # kT duplicated on partitions 0-63 and 64-127 so the two q heads'
        # QK matmuls sit in different PE row-groups.
        kT = qkpool.tile([128, T], bf16)
        if fp8_qk:
            qT8 = qkpool.tile([32, 2, 2, T], fp8)  # (head, d-group, token)
            kT8 = qkpool.tile([32, 2, T], fp8)

        # stage the very first QK's inputs before anything else: the first
        # exp is gated by DMA pipe-fill latency
        seg00, seg01 = segments[0]
        nc.sync.dma_start(
            qT[0:64, seg00 : seg00 + 512], qT_d[0:64, seg00 : seg00 + 512]
        )
        nc.sync.dma_start(
            kT[0:64, seg00 : seg00 + 128], kT_d[:, seg00 : seg00 + 128]
        )

        ident = const.tile([128, 128], f32)
        make_identity(nc, ident)
        # trimask[p, c] = 1 if c >= p else 0 (valid = query col >= key partition)
        trimask = const.tile([128, 128], bf16)
        nc.gpsimd.memset(trimask, 0.0)
        nc.gpsimd.affine_select(
            out=trimask,
            in_=trimask,
            compare_op=mybir.AluOpType.is_gt,
            fill=1.0,
            base=0,
            pattern=[[-1, 128]],
            channel_multiplier=1,
        )
        # additive-mask pair for the tail unit: maskW[r, c] = 1 iff c > r,
        # negI = -2000 * I. maskW.T @ negI adds -2000 above the diagonal of a
        # PSUM score block (exp(0.125 * -2000) == 0 in bf16), so the tail's
        # causal masking rides the PE instead of the post-exp DVE path.
        maskw = const.tile([128, 128], bf16)
        nc.gpsimd.memset(maskw, 0.0)
        nc.gpsimd.affine_select(
            out=maskw,
            in_=maskw,
            compare_op=mybir.AluOpType.is_gt,
            fill=1.0,
            base=1,
            pattern=[[-1, 128]],
            channel_multiplier=1,
        )
        negi = const.tile([128, 128], bf16)
        nc.vector.tensor_scalar_mul(negi, ident, -2000.0)

        # PE p-state warmup: the tensor engine drops to 1/2..1/3.7 speed after
        # idling and needs 3us of continuous work to reach full clock. A
        # dependency-free dummy matmul chain into the (yet unused) oT_0 bank
        # keeps it ramping while the first input DMAs are still in flight.
        warm_ps = ps_o.tile([128, 128], f32, tag="oT_0", name="pe_warm")

        def pe_warm(n):
            for _ in range(n):
                nc.tensor.matmul(
                    warm_ps[:, :128], trimask, trimask, start=True, stop=True
                )

        pe_warm(14)

        # ---------- units: one per (segment, q-head) ----------
        units = []
        for si, (s0, s1) in enumerate(segments):
            L = s1 - s0
            assert L <= 1024, "single-super kernel: segment lengths must be <=1024"
            kend = L
            nkt = ceil(kend / 128)
            kts = []
            for kt in range(nkt):
                klo = 128 * kt
                kp = min(128, kend - klo)
                kts.append({"klo": klo, "kp": kp, "c0": klo, "span": kend - klo})
            bins = _pack_bins(kts)
            edges = list(range(0, L, 512)) + [L]
            nblk = len(edges) - 1
            tchunk = None
            ctiles = None
            if L == 1024 and nkt == 8 and TCHUNK:
                # query cols [512, L) are computed TRANSPOSED on the PE
                # (out[q, d+1] = se.T @ [v|1], FD=65 per key tile): no PSUM
                # copy, no PE transpose, no block-close machinery for the
                # second half of each unit
                edges = [0, 512]
                nblk = 1
                tchunk = 512
                ctiles = [("oT_1", 0, 4)]
            for h in range(G):
                units.append(
                    {
                        "si": si,
                        "s0": s0,
                        "s1": s1,
                        "L": L,
                        "h": h,
                        "kts": kts,
                        "bins": bins,
                        "edges": edges,
                        "nblk": nblk,
                        "tchunk": tchunk,
                        "ctiles": ctiles,
                    }
                )

        # First unit: lead with a tiny 512-col bin so the first exp fires as
        # soon as one QK matmul lands (DMA pipe-fill dominates the start).
        fu = units[0]
        if fu["L"] == 1024 and len(fu["kts"]) == 8:
            fu["bins"] = [
                (512, [(0, 0, 512, 0)]),
                (1024, [(0, 512, 1024, 0), (4, 0, 512, 512)]),
                (1536, [(1, 0, 896, 0), (3, 0, 640, 896)]),
                (1536, [(2, 0, 768, 0), (5, 0, 384, 768), (6, 0, 256, 1152), (7, 0, 128, 1408)]),
            ]
        # Last unit: 4 smaller bins ordered so block 0 closes one exp before
        # the end and block 1's tail PV is minimal -- shrinks the serial
        # post-exp tail of the whole kernel.
        lu = units[-1]
        if lu["L"] == 1024 and len(lu["kts"]) == 8 and TCHUNK:
            lu["bins"] = [
                (1536, [(0, 0, 1024, 0), (4, 0, 512, 1024)]),
                (1536, [(1, 0, 896, 0), (3, 0, 640, 896)]),
                (1408, [(2, 0, 768, 0), (5, 0, 384, 768), (6, 0, 256, 1152)]),
                (128, [(7, 0, 128, 0)]),
            ]
            # the tail unit computes ALL its output transposed (8 chunks over
            # two PSUM tiles): no block-close machinery after the last exp --
            # just kt7's 8 transposed-PV pieces and one small finalize
            lu["tchunk"] = 0
            lu["edges"] = [0]
            lu["nblk"] = 0
            lu["ctiles"] = [("oT_0", 0, 2), ("oT_0", 2, 4), ("oT_1", 4, 8)]


        # global bin list; each entry: (ui, bi, used, members, share)
        # share = None (own sc tile) or (owner_gi, col_off) when several
        # small bins pack into one sc tile (tail: no WAR chain between them)
        gbins = []
        for ui, u in enumerate(units):
            sf = u.get("share_from")
            own_gi = None
            off = 0
            for bi, (used, members) in enumerate(u["bins"]):
                share = None
                if sf is not None and bi >= sf:
                    if bi == sf:
                        own_gi = len(gbins)
                    else:
                        share = (own_gi, off)
                    off += used
                gbins.append((ui, bi, used, members, share))

        # per (unit, block): first/last PV writer in emission order; per
        # (unit, chunk): ordered writer lists for chunk-major emission (PSUM
        # allows only one open accumulation group per 2KB zero region, so
        # chunks accumulate strictly one after another)
        first_w = {}
        last_w = {}
        chunk_w = {}
        for gi, (ui, bi, used, members, _share) in enumerate(gbins):
            u = units[ui]
            tch = u.get("tchunk")
            for kt, rlo, rhi, off in members:
                c0 = u["kts"][kt]["c0"]
                for m in range(u["nblk"]):
                    b0, b1 = u["edges"][m], u["edges"][m + 1]
                    if max(b0, c0 + rlo) < min(b1, c0 + rhi):
                        if (ui, m) not in first_w:
                            first_w[(ui, m)] = (bi, kt, rlo)
                        last_w[(ui, m)] = (bi, kt, rlo)
                if tch is not None:
                    nch = (u["L"] - tch) // 128
                    for c in range(nch):
                        cl, ch_ = tch + 128 * c, tch + 128 * c + 128
                        if max(cl, c0 + rlo) < min(ch_, c0 + rhi):
                            assert c0 + rlo <= cl and c0 + rhi >= ch_, (
                                "chunk coverage must be all-or-nothing"
                            )
                            chunk_w.setdefault((ui, c), []).append(
                                (gi, kt, rlo, off)
                            )

        dma_done = set()

        def emit_seg_dma(si):
            if si in dma_done or si >= len(segments):
                return
            dma_done.add(si)
            s0, s1 = segments[si]
            # kT/v ride the SP HWDGE queue; qT rides the Activation HWDGE
            # queue at the start (idle until the first exp, and the two
            # first-QK inputs then transfer in parallel) and the SWDGE
            # mid-kernel (Pool is idle; Act SEQ must stay clear for exp).
            kend = s1 - s0
            nkt = ceil(kend / 128)
            vst = vpool.tile([128, nkt, D + 1], bf16, tag="vt", name=f"vst_{si}")
            nfull = kend // 128

            def v_dma(vq):
                if nfull:
                    vq.dma_start(
                        vst[:, :nfull, 0:D],
                        v_d[s0 : s0 + nfull * 128, :].rearrange(
                            "(n p) d -> p n d", p=128
                        ),
                    )
                if kend % 128:
                    rem = kend % 128
                    vq.dma_start(
                        vst[:rem, nfull, 0:D], v_d[s0 + nfull * 128 : s0 + kend, :]
                    )

            if si == 0:
                # all of segment 0 rides the sync queue (the other queues'
                # first transfers race the launch), ordered by first use:
                # (kT 128 cols + qT 512 cols staged in the preamble), then
                # bin1's needs, then V for the first PV, then head 1
                nc.sync.dma_start(
                    qT[0:64, s0 + 512 : s1], qT_d[0:64, s0 + 512 : s1]
                )
                nc.sync.dma_start(
                    kT[0:64, s0 + 128 : s0 + 640], kT_d[:, s0 + 128 : s0 + 640]
                )
                nc.sync.dma_start(kT[0:64, s0 + 640 : s1], kT_d[:, s0 + 640 : s1])
                v_dma(nc.sync)
                nc.sync.dma_start(kT[64:128, s0:s1], kT_d[:, s0:s1])
                nc.sync.dma_start(qT[64:128, s0:s1], qT_d[64:128, s0:s1])
            else:
                v_dma(nc.sync)
                # segment 1's transfers ride the (backlogged) sync queue so
                # they can't jump ahead of segment 0's critical transfers on
                # the shared DMA engines; later segments use the idle SWDGE
                qq = nc.sync if si == 1 else nc.gpsimd
                qq.dma_start(qT[:, s0:s1], qT_d[:, s0:s1])
                nc.sync.dma_start(kT[0:64, s0:s1], kT_d[:, s0:s1])
                nc.sync.dma_start(kT[64:128, s0:s1], kT_d[:, s0:s1])
            nc.gpsimd.memset(vst[:, :, D : D + 1], 1.0)
            vst_by_seg[si] = vst
            if fp8_qk and si > 0:
                nc.sync.dma_start(
                    kT8[:, :, s0:s1],
                    kT8_d.rearrange("p (i t) -> p i t", i=2)[:, :, s0:s1],
                )
                q8q = nc.sync if si == 1 else nc.scalar
                q8q.dma_start(
                    qT8[:, :, :, s0:s1],
                    qT8_d.rearrange("p (h i t) -> p h i t", h=2, i=2)[
                        :, :, :, s0:s1
                    ],
                )

        vst_by_seg = {}

        # deferred finalize continuations, two phases so neither the PE nor
        # the DVE critical ops ever sit behind a waiting finalize op:
        #   tp-phase (PE transposes)  -> flushed before PV of the next bin
        #   norm-phase (DVE rcp/mul + out DMA) -> flushed one bin later
        def_tp = []
        def_norm = []

        def finalize_tp(ui, m, t65, t_lo, t_hi, row_off, uniq, tail=False, sc_space=False):
            """Transpose t65 cols [t_lo, t_hi) into PSUM tp; the rows land at
            block offset row_off in the output. Tail tps live in the score
            banks (dead by then) so the two stage chains don't serialize on
            the oT bank."""
            nchunk = ceil((t_hi - t_lo) / 128)
            if sc_space:
                tp = ps_sc.tile([128, 65 * nchunk], f32, tag="sc", name=f"tp_{uniq}")
            else:
                tp = ps_o.tile([128, 65 * nchunk], f32, tag=f"oT_{m}", name=f"tp_{uniq}")
            for j in range(nchunk):
                n = min(128, t_hi - t_lo - j * 128)
                nc.tensor.transpose(
                    tp[:n, 65 * j : 65 * j + 65],
                    t65[:65, t_lo + j * 128 : t_lo + j * 128 + n],
                    ident[:65, :65],
                )
            if tail:
                finalize_norm(ui, m, tp, t_lo, t_hi, row_off, uniq, tail=True)
            else:
                def_norm.append((ui, m, tp, t_lo, t_hi, row_off, uniq))

        def finalize_norm(ui, m, tp, t_lo, t_hi, row_off, uniq, tail=False):
            """Normalize transposed chunks and DMA the rows out."""
            u = units[ui]
            s0, h = u["s0"], u["h"]
            b0 = u["edges"][m]
            c_lo, c_hi = t_lo, t_hi
            nchunk = ceil((c_hi - c_lo) / 128)
            rcp = opool.tile([128, nchunk], f32, tag="rcp", name=f"rcp_{uniq}")
            osb = opool.tile([128, D * nchunk], f32, tag="osb", name=f"osb_{uniq}")
            nf = (c_hi - c_lo) // 128
            if nf:
                nc.vector.reciprocal(rcp[:, :nf], tp[:, D : 65 * nf : 65])
                tp_v = tp[:, : 65 * nf].rearrange("p (c k) -> p c k", k=65)[:, :, 0:D]
                rcp_v, tp_v = bass.broadcast_tensor_aps(
                    rcp[:, :nf].rearrange("p (c k) -> p c k", k=1), tp_v
                )
                nc.vector.tensor_mul(
                    osb[:, : D * nf].rearrange("p (c k) -> p c k", k=D), tp_v, rcp_v
                )
            if nf < nchunk:
                n = (c_hi - c_lo) - nf * 128
                nc.vector.reciprocal(
                    rcp[:n, nf : nf + 1], tp[:n, 65 * nf + D : 65 * nf + D + 1]
                )
                nc.vector.tensor_scalar_mul(
                    osb[:n, D * nf : D * nf + D],
                    tp[:n, 65 * nf : 65 * nf + D],
                    rcp[:n, nf : nf + 1],
                )
            r0 = s0 + b0 + row_off
            span = c_hi - c_lo
            cfull = span // 128
            # out DMAs ride the SWDGE (Pool) queue: Pool SEQ/engine are idle,
            # keeping the HWDGE/SP path free for input prefetch. The last
            # unit's go out on the (by then idle) SP HWDGE queue, which has
            # ~500ns less fixed latency per DMA.
            dmae = nc.sync if tail else nc.gpsimd
            if cfull:
                dmae.dma_start(
                    o_d[r0 : r0 + cfull * 128, D * h : D * h + D].rearrange(
                        "(c p) k -> p c k", p=128
                    ),
                    osb.rearrange("p (c k) -> p c k", k=D)[:, :cfull, :],
                )
            if span % 128:
                n = span % 128
                dmae.dma_start(
                    o_d[r0 + cfull * 128 : r0 + span, D * h : D * h + D],
                    osb[:n, D * cfull : D * cfull + D],
                )

        # ---------- main loop over global bins ----------
        sc_of = {}  # gi -> (sc tile, rows)
        se_of = {}  # gi -> se tile
        oT_of = {}  # (ui, m) -> oT psum tile

        def emit_qk(gi, premask=False):
            ui, bi, used, members, share = gbins[gi]
            u = units[ui]
            s0, h = u["s0"], u["h"]
            rows = 128 if len(members) > 1 else u["kts"][members[0][0]]["kp"]
            if share is not None:
                sc = sc_of[share[0]][0]
                coff = share[1]
            else:
                sc = ps_sc.tile([128, BIN_CAP], f32, tag="sc", name=f"sc_{ui}_{bi}")
                coff = 0
            sc_of[gi] = (sc, rows, coff)
            for kt, rlo, rhi, off in members:
                e = u["kts"][kt]
                klo, kp, c0 = e["klo"], e["kp"], e["c0"]
                is_diag = rlo == 0 and c0 == klo and kp == 128
                diag = premask and is_diag
                # pieces are cut so that no matmul output crosses a 512-f32
                # PSUM bank boundary of the sc tile (hardware restriction);
                # DoubleRow additionally caps at 256 cols (moving free <=512)
                tbase = coff + off - (c0 + rlo)  # tile col of query col 0
                pieces = []  # (a0, b1, kind)
                p0 = c0 + rlo
                use_fp8 = fp8_qk and kp == 128 and u["si"] > 0

                def cut(a0, hi, width, kind):
                    while a0 < hi:
                        bank_end = a0 + (512 - (tbase + a0) % 512)
                        b1 = min(a0 + width, hi, bank_end)
                        pieces.append((a0, b1, kind))
                        a0 = b1

                if diag and is_diag:
                    # premask: the first 128 cols pair a -2000 above-diagonal
                    # bias matmul with the QK in one accumulation group
                    pieces.append((c0, c0 + min(128, rhi), "mdiag"))
                    p0 = c0 + min(128, rhi)
                if use_fp8:
                    if is_diag:
                        # keep a wider bf16 leading strip: the first keys'
                        # softmax has few terms to average fp8 noise over
                        dn = min(384, rhi)
                        cut(p0, c0 + dn, 512, "diag")
                        p0 = c0 + dn
                    cut(p0, c0 + rhi, 256, "fp8")
                else:
                    cut(p0, c0 + rhi, 512, "diag")
                for a0, b1, kind in pieces:
                    lo = coff + off + a0 - (c0 + rlo)
                    if kind == "fp8":
                        nc.tensor.matmul(
                            sc[:kp, lo : lo + b1 - a0],
                            kT8[:, :, s0 + klo : s0 + klo + kp],
                            qT8[:, h, :, s0 + a0 : s0 + b1],
                            start=True,
                            stop=True,
                            perf_mode=mybir.MatmulPerfMode.DoubleRow,
                        )
                    elif kind == "mdiag":
                        # bias opens the group (a later start in this psum
                        # zero region lazily zeroes only its own bytes; these
                        # stay read-valid once the group closes)
                        nc.tensor.matmul(
                            sc[:kp, lo : lo + b1 - a0],
                            maskw[:, :kp],
                            negi[:, : b1 - a0],
                            start=True,
                            stop=False,
                            skip_group_check=True,
                        )
                        nc.tensor.matmul(
                            sc[:kp, lo : lo + b1 - a0],
                            kT[64 * h : 64 * h + 64, s0 + klo : s0 + klo + kp],
                            qT[64 * h : 64 * h + 64, s0 + a0 : s0 + b1],
                            start=False,
                            stop=True,
                            skip_group_check=True,
                            tile_position=(64 * h, 0),
                        )
                    else:
                        nc.tensor.matmul(
                            sc[:kp, lo : lo + b1 - a0],
                            kT[64 * h : 64 * h + 64, s0 + klo : s0 + klo + kp],
                            qT[64 * h : 64 * h + 64, s0 + a0 : s0 + b1],
                            start=True,
                            stop=True,
                            tile_position=(64 * h, 0),
                        )

        def emit_exp(gi):
            ui, bi, used, members, share = gbins[gi]
            sc, rows, coff = sc_of[gi]
            se = sepool.tile([128, BIN_CAP], bf16, tag="se", name=f"se_{ui}_{bi}")
            se_of[gi] = se
            nc.scalar.activation(
                se[:rows, :used],
                sc[:rows, coff : coff + used],
                mybir.ActivationFunctionType.Exp,
                scale=SCALE,
            )

        def emit_mask(gi, premask=False):
            if premask:
                return
            ui, bi, used, members, _share = gbins[gi]
            u = units[ui]
            se = se_of[gi]
            for kt, rlo, rhi, off in members:
                e = u["kts"][kt]
                if rlo == 0 and e["c0"] == e["klo"]:  # diagonal: causal mask
                    dn = min(e["kp"], rhi - rlo)
                    nc.vector.tensor_mul(
                        se[: e["kp"], off : off + dn],
                        se[: e["kp"], off : off + dn],
                        trimask[: e["kp"], :dn],
                    )

        def emit_pv(gi, tail_ui=None):
            ui, bi, used, members, _share = gbins[gi]
            u = units[ui]
            se = se_of[gi]
            vst = vst_by_seg[u["si"]]
            is_tail = ui == tail_ui
            nkt = len(u["kts"])
            tch = u.get("tchunk")
            for kt, rlo, rhi, off in members:
                e = u["kts"][kt]
                klo, kp, c0 = e["klo"], e["kp"], e["c0"]
                for m in range(u["nblk"]):
                    b0, b1 = u["edges"][m], u["edges"][m + 1]
                    a0 = max(b0, c0 + rlo)
                    a1 = min(b1, c0 + rhi)
                    if tch is not None:
                        a1 = min(a1, tch)
                    if a0 >= a1:
                        continue
                    blen = b1 - b0
                    key = (ui, m)
                    if key not in oT_of:
                        oT_of[key] = ps_o.tile(
                            [65, blen], f32, tag=f"oT_{m}", name=f"oT_{ui}_{m}"
                        )
                    oT = oT_of[key]
                    first = first_w[key] == (bi, kt, rlo)
                    last = last_w[key] == (bi, kt, rlo)
                    # two-stage close of the very last block: stop early on
                    # the penultimate key tile so most of the block can be
                    # finalized while the last 128-col PV still runs
                    tail2 = (
                        is_tail and m == u["nblk"] - 1 and nkt >= 2
                        and last_w[key][1] == nkt - 1
                    )
                    c_split = 128 * (nkt - 1) - b0
                    split2 = tail2 and 128 <= c_split < blen
                    stage1_here = split2 and kt == nkt - 2 and last_w[key] != (bi, kt, rlo) and a1 == c0 + rhi
                    if split2 and kt == nkt - 1:
                        # the final key tile accumulates into a score-bank
                        # tile so neither it nor the stage-1 copy WAR-block
                        # each other; stage 2 merges with a tensor_add
                        oTb = ps_sc.tile(
                            [65, a1 - a0], f32, tag="sc", name=f"oTb_{ui}_{m}"
                        )
                        u["tail_oTb"] = oTb
                        nc.tensor.matmul(
                            oTb[:, : a1 - a0],
                            vst[:, kt, :][:kp, : D + 1],
                            se[:kp, off + a0 - (c0 + rlo) : off + a1 - (c0 + rlo)],
                            start=True,
                            stop=True,
                        )
                    else:
                        nc.tensor.matmul(
                            oT[:, a0 - b0 : a1 - b0],
                            vst[:, kt, :][:kp, : D + 1],
                            se[:kp, off + a0 - (c0 + rlo) : off + a1 - (c0 + rlo)],
                            start=first,
                            stop=last or stage1_here,
                        )
                    if stage1_here:
                        # kt7 writes its own accumulator, so the stage-1 copy
                        # can run immediately without blocking anything
                        uq1 = f"{ui}_{m}_s1"
                        t65s = tpool.tile(
                            [65, 512], f32, tag="t65", name=f"t65_{uq1}"
                        )
                        nc.vector.tensor_copy(t65s[:, :c_split], oT[:, :c_split])
                        finalize_tp(ui, m, t65s, 0, c_split, 0, uq1,
                                    tail=True, sc_space=True)
                    elif last:
                        uniq = f"{ui}_{m}"
                        if split2:
                            # stage 2: merge main + last-tile accumulators
                            oTb = u["tail_oTb"]
                            t65b = tpool.tile(
                                [65, blen - c_split], f32, tag="t65b",
                                name=f"t65_{uniq}_s2"
                            )
                            nc.vector.tensor_add(
                                t65b[:, : blen - c_split],
                                oT[:, c_split:blen],
                                oTb[:, : blen - c_split],
                            )
                            finalize_tp(ui, m, t65b, 0, blen - c_split, c_split,
                                        uniq + "_s2", tail=True, sc_space=True)
                        else:
                            t65 = tpool.tile(
                                [65, blen], f32, tag="t65", name=f"t65_{uniq}"
                            )
                            nc.vector.tensor_copy(t65[:, :blen], oT[:, :blen])
                            if is_tail:
                                finalize_tp(ui, m, t65, 0, blen, 0, uniq, tail=True)
                            else:
                                def_tp.append((ui, m, t65, 0, blen, 0, uniq))

        tail_ui = gbins[-1][0]

        chunk_state = {}  # (ui, tile_idx) -> [next_chunk_ptr, finalized_ptr]

        def emit_chunks(gi):
            """Chunk-major transposed-PV emission: within a PSUM tile, chunk
            c's pieces (one per key tile, FD=65) emit once every earlier
            chunk is fully accumulated and all writers' se are available
            (PSUM allows one open accumulation group per zero region).
            A tile is normalized+DMAd when complete -- or early, once only
            its final chunk remains, so the tail's last chunk is the only
            post-stream work."""
            ui = gbins[gi][0]
            u = units[ui]
            tch = u.get("tchunk")
            if tch is None:
                return
            vst = vst_by_seg[u["si"]]
            is_tail = ui == len(units) - 1
            for ti, (tag, t_lo, t_hi) in enumerate(u["ctiles"]):
                st = chunk_state.setdefault((ui, ti), [t_lo, t_lo])
                while st[0] < t_hi:
                    c = st[0]
                    w = chunk_w[(ui, c)]
                    if w[-1][0] > gi:
                        break  # some writer's se not available yet
                    key = (ui, "t", ti)
                    if key not in oT_of:
                        oT_of[key] = ps_o.tile(
                            [128, t_hi - t_lo, D + 1], f32, tag=tag,
                            name=f"oTt_{ui}_{ti}",
                        )
                    oTt = oT_of[key]
                    cl = tch + 128 * c
                    for k, (wgi, kt, rlo, off) in enumerate(w):
                        e = u["kts"][kt]
                        scol = off + cl - (e["c0"] + rlo)
                        nc.tensor.matmul(
                            oTt[:, c - t_lo, :][:128, : D + 1],
                            se_of[wgi][: e["kp"], scol : scol + 128],
                            vst[:, kt, :][: e["kp"], : D + 1],
                            start=(k == 0),
                            stop=(k == len(w) - 1),
                        )
                    st[0] += 1
                if st[1] < st[0] and st[0] == t_hi:
                    finalize_chunks(ui, ti, st[1], st[0], tail=is_tail)
                    st[1] = st[0]

        def finalize_chunks(ui, ti, c_lo, c_hi, tail=False):
            """Normalize + DMA chunks [c_lo, c_hi) of unit ui's transposed
            region straight out of PSUM (no copies, no transposes)."""
            u = units[ui]
            tch, s0, h = u["tchunk"], u["s0"], u["h"]
            ncc = c_hi - c_lo
            t_lo = u["ctiles"][ti][1]
            oTt = oT_of[(ui, "t", ti)]
            c_lo, c_hi = c_lo - t_lo, c_hi - t_lo
            uniq = f"t_{ui}_{ti}_{c_lo}"
            rcpt = opool.tile([128, ncc], f32, tag="rcp", name=f"rcp_{uniq}")
            osbt = opool.tile([128, ncc * D], f32, tag="osb", name=f"osb_{uniq}")
            nc.vector.reciprocal(
                rcpt[:, :ncc].rearrange("p (c k) -> p c k", k=1),
                oTt[:, c_lo:c_hi, D : D + 1],
            )
            tp_v = oTt[:, c_lo:c_hi, 0:D]
            rcp_v, tp_v = bass.broadcast_tensor_aps(
                rcpt[:, :ncc].rearrange("p (c k) -> p c k", k=1), tp_v
            )
            nc.vector.tensor_mul(
                osbt.rearrange("p (c k) -> p c k", k=D), tp_v, rcp_v
            )
            r0 = s0 + tch + 128 * (t_lo + c_lo)
            dmae = nc.sync if tail else nc.gpsimd
            dmae.dma_start(
                o_d[r0 : r0 + 128 * ncc, D * h : D * h + D].rearrange(
                    "(c p) k -> p c k", p=128
                ),
                osbt.rearrange("p (c k) -> p c k", k=D),
            )

        qk_done = set()

        def qk_with_dma(gi):
            if gi in qk_done:
                return
            qk_done.add(gi)
            ui, bi, used, members, share = gbins[gi]
            u = units[ui]
            if bi == 0 and u["h"] == 0:
                emit_seg_dma(u["si"])
            elif bi == 1 and u["h"] == 1:
                # prefetch the next segment one bin into the second head so
                # its transfers don't contend with this segment's critical
                # ones on the shared DMA engines
                emit_seg_dma(u["si"] + 1)
            emit_qk(gi, premask=(PREMASK and ui == tail_ui))
            if gi < 6:
                # early bins: pad the PE pipeline between sparse QKs so the
                # p-state ramp survives the DMA waits
                pe_warm(4)
            if share is None and gi + 1 < GN and gbins[gi + 1][4] is not None:
                # sc-sharing group: emit all sharers' QKs in one early batch
                j = gi + 1
                while j < GN and gbins[j][4] is not None:
                    qk_with_dma(j)
                    j += 1

        # QK runs two bins ahead of exp so exp(g) never waits on PE: while
        # exp(g) streams, PE does QK(g+2) (gated only on the WAR release of
        # exp(g)'s score buffer) and then PV(g).
        GN = len(gbins)
        qk_with_dma(0)
        qk_with_dma(1)
        for gi in range(GN):
            emit_exp(gi)
            if gi + 2 < GN:
                qk_with_dma(gi + 2)
            emit_mask(gi, premask=(PREMASK and gbins[gi][0] == tail_ui))
            # tp-phase of bins closed earlier (PE, before PV so the oT psum
            # tag order stays oT(u) -> tp(u) -> oT(u+1)); at most one per
            # iteration to spread PE transpose load across exp periods, but
            # never hold one past its unit boundary (next unit's PV would
            # overwrite the oT bank the tp reads... order must be preserved)
            if def_tp:
                finalize_tp(*def_tp.pop(0))
            while def_tp and def_tp[0][0] < gbins[gi][0]:
                finalize_tp(*def_tp.pop(0))
            emit_pv(gi, tail_ui=(tail_ui if gbins[gi][0] == tail_ui else None))
            emit_chunks(gi)
            # norm-phase (DVE + out DMA) of tp-phases flushed last iteration
            while def_norm and def_norm[0][0] < gbins[gi][0]:
                finalize_norm(*def_norm.pop(0))
        while def_tp:
            finalize_tp(*def_tp.pop(0), tail=True)
        while def_norm:
            finalize_norm(*def_norm.pop(0), tail=True)

    nc.compile()
    return nc


def _segments_from_cu(cu_seqlens, T):
    edges = sorted(set([0, T] + [int(c) for c in cu_seqlens if 0 < int(c) < T]))
    return [(edges[i], edges[i + 1]) for i in range(len(edges) - 1)]


def kernel(q, k, v, k_cache, v_cache, slot_mapping, cu_seqlens):
    global LAST_RESULT
    T = q.shape[0]
    nslots = k_cache.shape[0]

    # Emulate scatter-then-gather through the paged cache: for duplicate slots
    # the last writer wins, so token i reads back k[lastw[slot[i]]].
    slot = np.asarray(slot_mapping, dtype=np.int64)
    lastw = np.zeros(nslots, dtype=np.int64)
    lastw[slot] = np.arange(T)
    lw = lastw[slot]
    k_eff = np.asarray(k)[lw]
    v_eff = np.asarray(v)[lw]

    segments = _segments_from_cu(np.asarray(cu_seqlens), T)
    key = (T, tuple(segments), FP8_QK, TCHUNK, PREMASK)
    if key not in _PROGRAM_CACHE:
        _PROGRAM_CACHE[key] = _build_program(T, segments)
    nc = _PROGRAM_CACHE[key]

    bf = ml_dtypes.bfloat16
    qhf = np.ascontiguousarray(
        np.asarray(q, dtype=np.float32).reshape(T, NKV * G, D).transpose(1, 2, 0)
    )  # [16, 64, T] f32
    khf = np.ascontiguousarray(k_eff.astype(np.float32).reshape(T, NKV, D).transpose(1, 2, 0))
    qh = qhf.astype(bf)
    kh = khf.astype(bf)
    vh = v_eff.reshape(T, NKV, D).astype(bf)  # [T, 8, 64]

    in_maps = [
        {
            "qT": np.ascontiguousarray(qh[2 * h : 2 * h + 2].reshape(128, T)),
            "kT": np.ascontiguousarray(kh[h]),
            "v": np.ascontiguousarray(vh[:, h, :]),
        }
        for h in range(NKV)
    ]
    if FP8_QK:
        f8 = ml_dtypes.float8_e4m3
        for h in range(NKV):
            # [32, head(2), d-group(2), T]: row (p, i) <-> d = 32*i + p
            q8 = qhf[2 * h : 2 * h + 2].reshape(2, 2, 32, T).transpose(2, 0, 1, 3)
            k8 = khf[h].reshape(2, 32, T).transpose(1, 0, 2)
            in_maps[h]["qT8"] = np.ascontiguousarray(q8).astype(f8).reshape(32, 4 * T)
            in_maps[h]["kT8"] = np.ascontiguousarray(k8).astype(f8).reshape(32, 2 * T)

    res = run_bass_kernel_spmd(nc, in_maps, core_ids=list(range(8)), trace=TRACE)
    LAST_RESULT = res

    out = np.empty((T, NKV * G * D), dtype=np.float32)
    ov = out.reshape(T, NKV, G * D)
    for h in range(NKV):
        ov[:, h, :] = res.results[h]["o"]
    return out
